# revision 17
# baseline (speedup 1.0000x reference)
"""CQAttention Trainium2 kernel.

Math (per batch b, H=256, q=2048, d=8192):
  Qp   = gelu(Q @ W.T + b)                       [q, H]
  S    = C @ Qp.T                                [d, q]
  P    = softmax(S, axis=q)
  out  = P @ Qp + C                              [d, H]

Sharding: data-parallel over batch, one batch per NeuronCore (8 cores).

Per-core pipeline (v3):
  - All transposes (W^T, Q^T, Qp natural, C^T) go through the DMA xbar
    (dma_start_transpose, fp16/bf16) instead of the PE: frees ~16k PE
    cycles/core and eliminates the identity matrix + the ~10us GpSimd
    startup stall entirely.  HW-validated out-AP patterns only: full-tile
    3D/4D outs (mid dims fold into the logical row index, mid-major) and
    3D outs with a strided middle at 128-multiple offsets.  2D outs into
    row-slices of a larger tile are silently corrupted by the xbar ucode.
  - Logits S^T tiles [q=128, d=512] with fp16 operands; two per PSUM
    pair-tile [128, 2, 512] so the ACT exp drains them in ONE instruction.
    Softmax without max-subtraction: |logits| < ~70 so fp32 exp is safe.
  - Attended accumulated over the 16 q-tiles into PSUM [d=128, 257] where
    column 256 is the row-sum (ones column on Qp); fused epilogue
    out = (attended * 1/rowsum) + C in one DVE op per tile.
  - Chunk pipeline: C loads 3 chunks ahead, C^T xbar-transposes 2 chunks
    ahead; the Q-side setup (linear+gelu per 4-tile group) is folded into
    chunk 0's step loop; attended lags logits by 3 steps.
"""

from contextlib import ExitStack

import numpy as np

import concourse.mybir as mybir
import concourse.tile as tile
from concourse import bacc
from concourse.bass_utils import run_bass_kernel_spmd

B, QL, D, H = 8, 2048, 8192, 256
N_CORES = 8
F32 = mybir.dt.float32
BF16 = mybir.dt.bfloat16
F16 = mybir.dt.float16

HC = H // 128      # feature chunks (2)
NQT = QL // 128    # q tiles (16)
NQG = NQT // 4     # q groups (4)
DC = 512           # d-chunk size
NDC = D // DC      # d chunks (16)
NDM = DC // 128    # d tiles per chunk (4)
QPW = 512          # qp row stride (padded; col 256 = ones)
LAG = 4            # attended lag behind logits (covers exp + qp-transpose
                   # latency and the previous chunk's epilogue PSUM drain)

AF = mybir.ActivationFunctionType
ALU = mybir.AluOpType


def build_body(ctx: ExitStack, tc: tile.TileContext, nc, Qd, Cd, Wd, bd, Od):
    singles = ctx.enter_context(tc.tile_pool(name="singles", bufs=1))
    qstat = ctx.enter_context(tc.tile_pool(name="qstat", bufs=1))
    cpool = ctx.enter_context(tc.tile_pool(name="cpool", bufs=5))
    ctpool = ctx.enter_context(tc.tile_pool(name="ctp", bufs=3))
    exppool = ctx.enter_context(tc.tile_pool(name="expp", bufs=2))
    outpool = ctx.enter_context(tc.tile_pool(name="outp", bufs=3))
    small = ctx.enter_context(tc.tile_pool(name="small", bufs=4))
    psum_l = ctx.enter_context(tc.tile_pool(name="psl", bufs=2, space="PSUM"))
    psum_a = ctx.enter_context(tc.tile_pool(name="psa", bufs=1, space="PSUM"))

    # --- C path: natural load, fp16 cast, C^T via one 4D xbar transpose ---
    def c_load(dc):
        c_nat = cpool.tile([128, NDM, H], F32, tag="cnat", name=f"cnat{dc}")
        nc.sync.dma_start(
            out=c_nat[:],
            in_=Cd[dc * DC:(dc + 1) * DC, :].rearrange("(a p) h -> p a h", p=128))
        c_bf = cpool.tile([128, NDM, H], F16, tag="cbf", name=f"cbf{dc}")
        nc.vector.tensor_copy(c_bf[:], c_nat[:])
        return c_nat, c_bf

    c_nats = {}
    cts = {}

    def c_prep(dc):
        # ct[h', dm, hc, d'] = C[dc*DC + dm*128 + d', hc*128 + h']
        ct = ctpool.tile([128, NDM, HC, 128], F16, tag="ct", name=f"ct{dc}")
        nc.scalar.dma_start_transpose(ct[:], c_nats[dc][1][:])
        cts[dc] = ct

    # --- setup: issue every DRAM load on Sync first (no inline waits),
    # casts on DVE in dependency-urgency order, xbar transposes on ACT's
    # separate hwdge queue so they never block the Sync load stream ---
    w_nat = singles.tile([128, HC, H], F32)  # [o in-chunk, om, h]
    nc.sync.dma_start(out=w_nat[:],
                      in_=Wd.rearrange("(a p) h -> p a h", p=128))
    bias = singles.tile([128, HC, 1], F32)
    nc.sync.dma_start(out=bias[:, :, 0], in_=bd.rearrange("(c p) -> p c", p=128))

    q_nat = cpool.tile([128, NQT, H], F32, tag="qnat", bufs=1)
    q_f16 = cpool.tile([128, NQT, H], F16, tag="qf16", bufs=1)
    q_view = Qd.rearrange("(a p) h -> p a h", p=128)
    for qg in range(NQT // 2):
        nc.sync.dma_start(out=q_nat[:, qg * 2:(qg + 1) * 2, :],
                          in_=q_view[:, qg * 2:(qg + 1) * 2, :])

    setup_cn = {}
    for _dc in range(4):
        cn = cpool.tile([128, NDM, H], F32, tag="cnat", name=f"cnat{_dc}")
        nc.sync.dma_start(
            out=cn[:],
            in_=Cd[_dc * DC:(_dc + 1) * DC, :].rearrange("(a p) h -> p a h", p=128))
        setup_cn[_dc] = cn

    # per-group tiles (full-tile xbar writes only)
    qts = [qstat.tile([128, 4, HC, 128], F16, name=f"qt{g}") for g in range(NQG)]
    qps = [qstat.tile([128, 4, QPW], BF16, name=f"qp{g}") for g in range(NQG)]
    qpt = qstat.tile([128, HC, QL], F16)
    qpt_bf = qstat.tile([128, HC, QL], BF16)
    for g in range(NQG):
        nc.vector.memset(qps[g][:, :, H:H + 1], 1.0)

    w_f16 = singles.tile([128, HC, H], F16, name="w_f16")
    nc.vector.tensor_copy(w_f16[:], w_nat[:])
    for qg in range(NQT // 2):
        nc.vector.tensor_copy(q_f16[:, qg * 2:(qg + 1) * 2, :],
                              q_nat[:, qg * 2:(qg + 1) * 2, :])
    for _dc in range(4):
        cb = cpool.tile([128, NDM, H], F16, tag="cbf", name=f"cbf{_dc}")
        nc.vector.tensor_copy(cb[:], setup_cn[_dc][:])
        c_nats[_dc] = (setup_cn[_dc], cb)

    # wt[h', om, hc, o'] = W[om*128+o', hc*128+h']
    wt = qstat.tile([128, HC, HC, 128], F16)
    nc.scalar.dma_start_transpose(wt[:], w_f16[:])

    def qt_prep(qg):
        # qt[h', k, hc, q'] = Q[(qg*4+k)*128 + q', hc*128 + h']
        nc.scalar.dma_start_transpose(qts[qg][:], q_f16[:, qg * 4:(qg + 1) * 4, :])

    def q_group(qg):
        qs = slice(qg * 512, (qg + 1) * 512)
        plin = psum_l.tile([128, 2, 512], F32, tag="pl", name=f"plin{qg}")
        for om in range(HC):
            for hc in range(HC):
                nc.tensor.matmul(
                    plin[:, om, :],
                    wt[:, om, hc, :],
                    qts[qg][:, :, hc, :],
                    start=(hc == 0),
                    stop=(hc == HC - 1),
                )
        for om in range(HC):
            nc.scalar.activation(qpt[:, om, qs], plin[:, om, :], AF.Gelu,
                                 bias=bias[:, om, :], scale=1.0)
            nc.scalar.activation(qpt_bf[:, om, qs], plin[:, om, :], AF.Gelu,
                                 bias=bias[:, om, :], scale=1.0)
        for om in range(HC):
            # qp[q', k, om*128+o'] = Qp[(qg*4+k)*128 + q', om*128+o']
            nc.scalar.dma_start_transpose(
                qps[qg][:, :, om * 128:(om + 1) * 128], qpt_bf[:, om, qs])

    qt_prep(0)
    c_prep(0)
    q_group(0)
    c_prep(1)

    for dc in range(NDC):
        c_nat = c_nats[dc][0]
        ct = cts[dc]
        expt = exppool.tile([128, NQT, DC], BF16)
        pa = [psum_a.tile([128, H + 1], F32, tag=f"a{dm}", name=f"pa{dm}")
              for dm in range(NDM)]
        plp = None
        for step in range(NQT + LAG):
            if dc == 0 and step in (1, 5, 9):
                qt_prep(step // 4 + 1)
            if dc == 0 and step in (4, 8, 12):
                q_group(step // 4)
            if step == 2 and dc + 3 < NDC:
                c_nats[dc + 3] = c_load(dc + 3)
            if step == 6 and dc + 2 < NDC and dc + 2 > 1:
                c_prep(dc + 2)
            if step < NQT:
                qi = step
                par = qi & 1
                if par == 0:
                    plp = psum_l.tile([128, 2, DC], F32, tag="pl",
                                      name=f"pl{dc}_{qi}")
                for hc in range(HC):
                    nc.tensor.matmul(
                        plp[:, par, :],
                        qpt[:, hc, qi * 128:(qi + 1) * 128],
                        ct[:, :, hc, :],
                        start=(hc == 0),
                        stop=(hc == HC - 1),
                    )
                if par == 1:
                    nc.scalar.activation(
                        expt[:, qi - 1:qi + 1, :], plp[:], AF.Exp)
            if step >= LAG:
                qj = step - LAG
                for dm in range(NDM):
                    nc.tensor.matmul(
                        pa[dm][:],
                        expt[:, qj, dm * 128:(dm + 1) * 128],
                        qps[qj // 4][:, qj % 4, 0:H + 1],
                        start=(qj == 0),
                        stop=(qj == NQT - 1),
                    )

        o_sb = outpool.tile([128, NDM, H], F32)
        for dm in range(NDM):
            rec = small.tile([128, 1], F32)
            nc.vector.reciprocal(rec[:], pa[dm][:, H:H + 1])
            nc.vector.scalar_tensor_tensor(
                o_sb[:, dm, :], pa[dm][:, 0:H], rec[:], c_nat[:, dm, :],
                ALU.mult, ALU.add,
            )
        nc.sync.dma_start(
            out=Od[dc * DC:(dc + 1) * DC, :].rearrange("(a p) h -> p a h", p=128),
            in_=o_sb[:])
        del c_nats[dc], cts[dc]


def build_nc():
    nc = bacc.Bacc("TRN2", target_bir_lowering=False, debug=False,
                   num_devices=N_CORES)
    Qd = nc.dram_tensor("Q", [QL, H], F32, kind="ExternalInput")
    Cd = nc.dram_tensor("C", [D, H], F32, kind="ExternalInput")
    Wd = nc.dram_tensor("W", [H, H], F32, kind="ExternalInput")
    bd = nc.dram_tensor("b", [H], F32, kind="ExternalInput")
    Od = nc.dram_tensor("out", [D, H], F32, kind="ExternalOutput")
    with tile.TileContext(nc) as tc:
        with ExitStack() as ctx:
            build_body(ctx, tc, nc, Qd[:], Cd[:], Wd[:], bd[:], Od[:])
    nc.finalize()
    return nc


_NC = None


def get_nc():
    global _NC
    if _NC is None:
        _NC = build_nc()
    return _NC


def kernel(Q, C, W, b):
    assert Q.shape == (B, QL, H) and C.shape == (B, D, H)
    nc = get_nc()
    in_maps = [
        {
            "Q": np.ascontiguousarray(Q[i], dtype=np.float32),
            "C": np.ascontiguousarray(C[i], dtype=np.float32),
            "W": np.ascontiguousarray(W, dtype=np.float32),
            "b": np.ascontiguousarray(b, dtype=np.float32),
        }
        for i in range(N_CORES)
    ]
    res = run_bass_kernel_spmd(nc, in_maps, core_ids=list(range(N_CORES)))
    return np.stack([res.results[i]["out"] for i in range(N_CORES)], axis=0)


# revision 19
# speedup vs baseline: 1.0214x; 1.0214x over previous
"""CQAttention Trainium2 kernel.

Math (per batch b, H=256, q=2048, d=8192):
  Qp   = gelu(Q @ W.T + b)                       [q, H]
  S    = C @ Qp.T                                [d, q]
  P    = softmax(S, axis=q)
  out  = P @ Qp + C                              [d, H]

Sharding: data-parallel over batch, one batch per NeuronCore (8 cores).

Per-core pipeline (v3):
  - All transposes (W^T, Q^T, Qp natural, C^T) go through the DMA xbar
    (dma_start_transpose, fp16/bf16) instead of the PE: frees ~16k PE
    cycles/core and eliminates the identity matrix + the ~10us GpSimd
    startup stall entirely.  HW-validated out-AP patterns only: full-tile
    3D/4D outs (mid dims fold into the logical row index, mid-major) and
    3D outs with a strided middle at 128-multiple offsets.  2D outs into
    row-slices of a larger tile are silently corrupted by the xbar ucode.
  - Logits S^T tiles [q=128, d=512] with fp16 operands; two per PSUM
    pair-tile [128, 2, 512] so the ACT exp drains them in ONE instruction.
    Softmax without max-subtraction: |logits| < ~70 so fp32 exp is safe.
  - Attended accumulated over the 16 q-tiles into PSUM [d=128, 257] where
    column 256 is the row-sum (ones column on Qp); fused epilogue
    out = (attended * 1/rowsum) + C in one DVE op per tile.
  - Chunk pipeline: C loads 3 chunks ahead, C^T xbar-transposes 2 chunks
    ahead; the Q-side setup (linear+gelu per 4-tile group) is folded into
    chunk 0's step loop; attended lags logits by 3 steps.
"""

from contextlib import ExitStack

import numpy as np

import concourse.mybir as mybir
import concourse.tile as tile
from concourse import bacc
from concourse.bass_utils import run_bass_kernel_spmd

B, QL, D, H = 8, 2048, 8192, 256
N_CORES = 8
F32 = mybir.dt.float32
BF16 = mybir.dt.bfloat16
F16 = mybir.dt.float16

HC = H // 128      # feature chunks (2)
NQT = QL // 128    # q tiles (16)
NQG = NQT // 4     # q groups (4)
DC = 512           # d-chunk size
NDC = D // DC      # d chunks (16)
NDM = DC // 128    # d tiles per chunk (4)
QPW = 512          # qp row stride (padded; col 256 = ones)
LAG = 4            # attended lag behind logits (covers exp + qp-transpose
                   # latency and the previous chunk's epilogue PSUM drain)

AF = mybir.ActivationFunctionType
ALU = mybir.AluOpType


def build_body(ctx: ExitStack, tc: tile.TileContext, nc, Qd, Cd, Wd, bd, Od):
    singles = ctx.enter_context(tc.tile_pool(name="singles", bufs=1))
    qstat = ctx.enter_context(tc.tile_pool(name="qstat", bufs=1))
    cpool = ctx.enter_context(tc.tile_pool(name="cpool", bufs=5))
    ctpool = ctx.enter_context(tc.tile_pool(name="ctp", bufs=3))
    exppool = ctx.enter_context(tc.tile_pool(name="expp", bufs=2))
    outpool = ctx.enter_context(tc.tile_pool(name="outp", bufs=3))
    small = ctx.enter_context(tc.tile_pool(name="small", bufs=4))
    psum_l = ctx.enter_context(tc.tile_pool(name="psl", bufs=2, space="PSUM"))
    psum_a = ctx.enter_context(tc.tile_pool(name="psa", bufs=1, space="PSUM"))

    # --- C path: natural load, fp16 cast, C^T via one 4D xbar transpose ---
    def c_load(dc):
        c_nat = cpool.tile([128, NDM, H], F32, tag="cnat", name=f"cnat{dc}")
        nc.sync.dma_start(
            out=c_nat[:],
            in_=Cd[dc * DC:(dc + 1) * DC, :].rearrange("(a p) h -> p a h", p=128))
        c_bf = cpool.tile([128, NDM, H], F16, tag="cbf", name=f"cbf{dc}")
        nc.vector.tensor_copy(c_bf[:], c_nat[:])
        return c_nat, c_bf

    c_nats = {}
    cts = {}

    def c_prep(dc):
        # ct[h', dm, hc, d'] = C[dc*DC + dm*128 + d', hc*128 + h']
        ct = ctpool.tile([128, NDM, HC, 128], F16, tag="ct", name=f"ct{dc}")
        nc.scalar.dma_start_transpose(ct[:], c_nats[dc][1][:])
        cts[dc] = ct

    # --- setup: issue every DRAM load on Sync first (no inline waits),
    # casts on DVE in dependency-urgency order, xbar transposes on ACT's
    # separate hwdge queue so they never block the Sync load stream ---
    # per-group tiles (full-tile xbar writes only)
    qts = [qstat.tile([128, 4, HC, 128], F16, name=f"qt{g}") for g in range(NQG)]
    qps = [qstat.tile([128, 4, QPW], BF16, name=f"qp{g}") for g in range(NQG)]
    qpt = qstat.tile([128, HC, QL], F16)
    qpt_bf = qstat.tile([128, HC, QL], BF16)
    for g in range(NQG):
        nc.vector.memset(qps[g][:, :, H:H + 1], 1.0)

    # Load order = critical-chain order: the SP DMA ring serializes
    # transfers, so W -> C0 -> Q(0:4) gates the first PE/exp work; the
    # rest trails behind it.
    w_nat = singles.tile([128, HC, H], F32)  # [o in-chunk, om, h]
    nc.sync.dma_start(out=w_nat[:],
                      in_=Wd.rearrange("(a p) h -> p a h", p=128))
    bias = singles.tile([128, HC, 1], F32)
    nc.sync.dma_start(out=bias[:, :, 0], in_=bd.rearrange("(c p) -> p c", p=128))

    q_nat = cpool.tile([128, NQT, H], F32, tag="qnat", bufs=1)
    q_f16 = cpool.tile([128, NQT, H], F16, tag="qf16", bufs=1)
    q_view = Qd.rearrange("(a p) h -> p a h", p=128)
    setup_cn = {}
    w_f16 = singles.tile([128, HC, H], F16, name="w_f16")

    def load_c(_dc):
        cn = cpool.tile([128, NDM, H], F32, tag="cnat", name=f"cnat{_dc}")
        nc.sync.dma_start(
            out=cn[:],
            in_=Cd[_dc * DC:(_dc + 1) * DC, :].rearrange("(a p) h -> p a h", p=128))
        setup_cn[_dc] = cn

    def load_q(qg):
        nc.sync.dma_start(out=q_nat[:, qg * 2:(qg + 1) * 2, :],
                          in_=q_view[:, qg * 2:(qg + 1) * 2, :])

    def cast_q(qg):
        nc.vector.tensor_copy(q_f16[:, qg * 2:(qg + 1) * 2, :],
                              q_nat[:, qg * 2:(qg + 1) * 2, :])

    def cast_c(_dc):
        cb = cpool.tile([128, NDM, H], F16, tag="cbf", name=f"cbf{_dc}")
        nc.vector.tensor_copy(cb[:], setup_cn[_dc][:])
        c_nats[_dc] = (setup_cn[_dc], cb)

    load_c(0)
    load_q(0)
    load_q(1)
    load_c(1)
    load_q(2)
    load_q(3)
    load_c(2)
    for qg in range(4, 8):
        load_q(qg)
    load_c(3)

    nc.vector.tensor_copy(w_f16[:], w_nat[:])
    cast_c(0)
    cast_q(0)
    cast_q(1)
    cast_c(1)
    cast_q(2)
    cast_q(3)
    cast_c(2)
    for qg in range(4, 8):
        cast_q(qg)
    cast_c(3)

    # wt[h', om, hc, o'] = W[om*128+o', hc*128+h']
    wt = qstat.tile([128, HC, HC, 128], F16)
    nc.scalar.dma_start_transpose(wt[:], w_f16[:])

    def qt_prep(qg):
        # qt[h', k, hc, q'] = Q[(qg*4+k)*128 + q', hc*128 + h']
        nc.scalar.dma_start_transpose(qts[qg][:], q_f16[:, qg * 4:(qg + 1) * 4, :])

    def q_group(qg):
        qs = slice(qg * 512, (qg + 1) * 512)
        plin = psum_l.tile([128, 2, 512], F32, tag="pl", name=f"plin{qg}")
        for om in range(HC):
            for hc in range(HC):
                nc.tensor.matmul(
                    plin[:, om, :],
                    wt[:, om, hc, :],
                    qts[qg][:, :, hc, :],
                    start=(hc == 0),
                    stop=(hc == HC - 1),
                )
        for om in range(HC):
            nc.scalar.activation(qpt[:, om, qs], plin[:, om, :], AF.Gelu,
                                 bias=bias[:, om, :], scale=1.0)
            nc.scalar.activation(qpt_bf[:, om, qs], plin[:, om, :], AF.Gelu,
                                 bias=bias[:, om, :], scale=1.0)
        for om in range(HC):
            # qp[q', k, om*128+o'] = Qp[(qg*4+k)*128 + q', om*128+o']
            nc.scalar.dma_start_transpose(
                qps[qg][:, :, om * 128:(om + 1) * 128], qpt_bf[:, om, qs])

    c_prep(0)
    qt_prep(0)
    q_group(0)
    c_prep(1)

    for dc in range(NDC):
        c_nat = c_nats[dc][0]
        ct = cts[dc]
        expt = exppool.tile([128, NQT, DC], BF16)
        pa = [psum_a.tile([128, H + 1], F32, tag=f"a{dm}", name=f"pa{dm}")
              for dm in range(NDM)]
        plp = None
        for step in range(NQT + LAG):
            if dc == 0 and step in (1, 5, 9):
                qt_prep(step // 4 + 1)
            if dc == 0 and step in (4, 8, 12):
                q_group(step // 4)
            if step == 2 and dc + 3 < NDC:
                c_nats[dc + 3] = c_load(dc + 3)
            if step == 6 and dc + 2 < NDC and dc + 2 > 1:
                c_prep(dc + 2)
            if step < NQT:
                qi = step
                par = qi & 1
                if par == 0:
                    plp = psum_l.tile([128, 2, DC], F32, tag="pl",
                                      name=f"pl{dc}_{qi}")
                for hc in range(HC):
                    nc.tensor.matmul(
                        plp[:, par, :],
                        qpt[:, hc, qi * 128:(qi + 1) * 128],
                        ct[:, :, hc, :],
                        start=(hc == 0),
                        stop=(hc == HC - 1),
                    )
                if par == 1:
                    nc.scalar.activation(
                        expt[:, qi - 1:qi + 1, :], plp[:], AF.Exp)
            if step >= LAG:
                qj = step - LAG
                for dm in range(NDM):
                    nc.tensor.matmul(
                        pa[dm][:],
                        expt[:, qj, dm * 128:(dm + 1) * 128],
                        qps[qj // 4][:, qj % 4, 0:H + 1],
                        start=(qj == 0),
                        stop=(qj == NQT - 1),
                    )

        o_sb = outpool.tile([128, NDM, H], F32)
        for dm in range(NDM):
            rec = small.tile([128, 1], F32)
            nc.vector.reciprocal(rec[:], pa[dm][:, H:H + 1])
            nc.vector.scalar_tensor_tensor(
                o_sb[:, dm, :], pa[dm][:, 0:H], rec[:], c_nat[:, dm, :],
                ALU.mult, ALU.add,
            )
        nc.sync.dma_start(
            out=Od[dc * DC:(dc + 1) * DC, :].rearrange("(a p) h -> p a h", p=128),
            in_=o_sb[:])
        del c_nats[dc], cts[dc]


def build_nc():
    nc = bacc.Bacc("TRN2", target_bir_lowering=False, debug=False,
                   num_devices=N_CORES)
    Qd = nc.dram_tensor("Q", [QL, H], F32, kind="ExternalInput")
    Cd = nc.dram_tensor("C", [D, H], F32, kind="ExternalInput")
    Wd = nc.dram_tensor("W", [H, H], F32, kind="ExternalInput")
    bd = nc.dram_tensor("b", [H], F32, kind="ExternalInput")
    Od = nc.dram_tensor("out", [D, H], F32, kind="ExternalOutput")
    with tile.TileContext(nc) as tc:
        with ExitStack() as ctx:
            build_body(ctx, tc, nc, Qd[:], Cd[:], Wd[:], bd[:], Od[:])
    nc.finalize()
    return nc


_NC = None


def get_nc():
    global _NC
    if _NC is None:
        _NC = build_nc()
    return _NC


def kernel(Q, C, W, b):
    assert Q.shape == (B, QL, H) and C.shape == (B, D, H)
    nc = get_nc()
    in_maps = [
        {
            "Q": np.ascontiguousarray(Q[i], dtype=np.float32),
            "C": np.ascontiguousarray(C[i], dtype=np.float32),
            "W": np.ascontiguousarray(W, dtype=np.float32),
            "b": np.ascontiguousarray(b, dtype=np.float32),
        }
        for i in range(N_CORES)
    ]
    res = run_bass_kernel_spmd(nc, in_maps, core_ids=list(range(N_CORES)))
    return np.stack([res.results[i]["out"] for i in range(N_CORES)], axis=0)


# revision 23
# speedup vs baseline: 1.0919x; 1.0691x over previous
"""CQAttention Trainium2 kernel.

Math (per batch b, H=256, q=2048, d=8192):
  Qp   = gelu(Q @ W.T + b)                       [q, H]
  S    = C @ Qp.T                                [d, q]
  P    = softmax(S, axis=q)
  out  = P @ Qp + C                              [d, H]

Sharding: data-parallel over batch, one batch per NeuronCore (8 cores).

Per-core pipeline (v6 = tuned baseline):
  - C^T via PE transposes (identity matmul) as in the baseline: the DMA
    xbar path for C^T loses ~30us to DMA-ring serialization + ACT FIFO
    head-of-line blocking (measured), PE transposes cost only ~7us.
  - W^T, Q^T, Qp-natural via DMA xbar transposes instead of the PE
    (setup-sized, off the critical path): frees ~7us of PE work and the
    whole setup_pt PSUM machinery.  HW-validated xbar out-AP patterns
    only: full-tile 3D/4D outs (mid dims fold into the logical row index,
    mid-major order) and 3D outs with 128-multiple strides/offsets.
  - Logits^T tiles [q=128, d=512] with fp16 operands (full PE rate,
    ~11-bit mantissa); exp on ACT straight from PSUM to bf16 (softmax
    without max-subtraction: |logits| < ~70 so fp32 exp is safe);
    attended accumulated over the 16 q-tiles into PSUM [d=128, 257]
    where column 256 is the row-sum (ones column on Qp); fused epilogue
    out = (attended * 1/rowsum) + C in one DVE op per tile.
  - Setup DMA order = critical-chain order (W, C0, Q0:4 first): the SP
    DMA ring serializes transfers at ~150-230 GB/s effective.
  - Chunk pipeline: C loads 3 chunks ahead, C transposes 2 chunks ahead,
    attended lags logits/exp by 3 steps; the Q-side setup is folded into
    chunk 0's step loop so the PE never idles at startup.
"""

from contextlib import ExitStack

import numpy as np

import concourse.mybir as mybir
import concourse.tile as tile
from concourse import bacc
from concourse.bass_utils import run_bass_kernel_spmd
from concourse.masks import make_identity

B, QL, D, H = 8, 2048, 8192, 256
N_CORES = 8
F32 = mybir.dt.float32
BF16 = mybir.dt.bfloat16
F16 = mybir.dt.float16

HC = H // 128      # feature chunks (2)
NQT = QL // 128    # q tiles (16)
NQG = NQT // 4     # q groups (4)
DC = 512           # d-chunk size
NDC = D // DC      # d chunks (16)
NDM = DC // 128    # d tiles per chunk (4)
QPW = 512          # qp row stride (xbar needs 128-multiple; col 256 = ones)
LAG = 3            # attended lag behind logits/exp

AF = mybir.ActivationFunctionType
ALU = mybir.AluOpType


def build_body(ctx: ExitStack, tc: tile.TileContext, nc, Qd, Cd, Wd, bd, Od):
    singles = ctx.enter_context(tc.tile_pool(name="singles", bufs=1))
    qstat = ctx.enter_context(tc.tile_pool(name="qstat", bufs=1))
    cpool = ctx.enter_context(tc.tile_pool(name="cpool", bufs=5))
    ctpool = ctx.enter_context(tc.tile_pool(name="ctp", bufs=3))
    exppool = ctx.enter_context(tc.tile_pool(name="expp", bufs=2))
    outpool = ctx.enter_context(tc.tile_pool(name="outp", bufs=3))
    small = ctx.enter_context(tc.tile_pool(name="small", bufs=4))
    psum_l = ctx.enter_context(tc.tile_pool(name="psl", bufs=2, space="PSUM"))
    psum_t = ctx.enter_context(tc.tile_pool(name="pst", bufs=2, space="PSUM"))
    psum_a = ctx.enter_context(tc.tile_pool(name="psa", bufs=1, space="PSUM"))

    ident = singles.tile([128, 128], F16)
    make_identity(nc, ident)

    # --- C path ---
    c_nats = {}
    cts = {}

    def c_load(dc, cast=True):
        c_nat = cpool.tile([128, NDM, H], F32, tag="cnat", name=f"cnat{dc}")
        nc.sync.dma_start(
            out=c_nat[:],
            in_=Cd[dc * DC:(dc + 1) * DC, :].rearrange("(a p) h -> p a h", p=128))
        if not cast:
            return c_nat
        c_bf = cpool.tile([128, NDM, H], F16, tag="cbf", name=f"cbf{dc}")
        nc.vector.tensor_copy(c_bf[:], c_nat[:])
        return c_nat, c_bf

    def c_cast(dc, c_nat):
        c_bf = cpool.tile([128, NDM, H], F16, tag="cbf", name=f"cbf{dc}")
        nc.vector.tensor_copy(c_bf[:], c_nat[:])
        return c_nat, c_bf

    def c_transpose(dc, c_bf, hc):
        pt = psum_t.tile([128, 512], F16, tag="pt", name=f"ptc{dc}_{hc}")
        for dm in range(NDM):
            nc.tensor.transpose(
                pt[:, dm * 128:(dm + 1) * 128],
                c_bf[:, dm, hc * 128:(hc + 1) * 128], ident[:])
        return pt

    def ct_alloc(dc):
        return ctpool.tile([128, HC, DC], F16, tag="ct", name=f"ct{dc}")

    def c_prep(dc):
        cts[dc] = ct_alloc(dc)
        for hc in range(HC):
            pt = c_transpose(dc, c_nats[dc][1], hc)
            nc.vector.tensor_copy(cts[dc][:, hc, :], pt[:])

    # --- static tiles ---
    w_nat = singles.tile([128, HC, H], F32)  # [o in-chunk, om, h]
    w_f16 = singles.tile([128, HC, H], F16, name="w_f16")
    # wt[h', om, hc, o'] = W[om*128+o', hc*128+h']
    wt = qstat.tile([128, HC, HC, 128], F16)
    bias = singles.tile([128, HC, 1], F32)
    q_nat = cpool.tile([128, NQT, H], F32, tag="qnat", bufs=1)
    q_f16 = cpool.tile([128, NQT, H], F16, tag="qf16", bufs=1)
    # qt[g][h', k, hc, q'] = Q[(4g+k)*128+q', hc*128+h']
    qts = [qstat.tile([128, 4, HC, 128], F16, name=f"qt{g}") for g in range(NQG)]
    # qp[g][q', k, col] = Qp[(4g+k)*128+q', col], col 256 = 1.0
    qps = [qstat.tile([128, 4, QPW], BF16, name=f"qp{g}") for g in range(NQG)]
    qpt = qstat.tile([128, HC, QL], F16)
    qpt_bf = qstat.tile([128, HC, QL], BF16)
    for g in range(NQG):
        nc.vector.memset(qps[g][:, :, H:H + 1], 1.0)

    # --- setup loads: critical-chain order on the serialized SP ring ---
    q_view = Qd.rearrange("(a p) h -> p a h", p=128)

    def load_q(qg):
        nc.sync.dma_start(out=q_nat[:, qg * 2:(qg + 1) * 2, :],
                          in_=q_view[:, qg * 2:(qg + 1) * 2, :])

    def cast_q(qg):
        nc.vector.tensor_copy(q_f16[:, qg * 2:(qg + 1) * 2, :],
                              q_nat[:, qg * 2:(qg + 1) * 2, :])

    nc.sync.dma_start(out=w_nat[:],
                      in_=Wd.rearrange("(a p) h -> p a h", p=128))
    nc.sync.dma_start(out=bias[:, :, 0], in_=bd.rearrange("(c p) -> p c", p=128))
    c_nats[0] = c_load(0)
    load_q(0)
    load_q(1)
    cn1 = c_load(1, cast=False)
    load_q(2)
    load_q(3)
    cn2 = c_load(2, cast=False)
    for qg in range(4, 8):
        load_q(qg)
    cn3 = c_load(3, cast=False)

    nc.vector.tensor_copy(w_f16[:], w_nat[:])
    cast_q(0)
    cast_q(1)
    c_nats[1] = c_cast(1, cn1)
    for qg in range(2, 8):
        cast_q(qg)
    c_nats[2] = c_cast(2, cn2)
    c_nats[3] = c_cast(3, cn3)

    # setup transposes on ACT's hwdge queue (idle pre-loop)
    nc.scalar.dma_start_transpose(wt[:], w_f16[:])

    def qt_prep(qg, eng):
        eng.dma_start_transpose(qts[qg][:], q_f16[:, qg * 4:(qg + 1) * 4, :])

    def q_group(qg):
        # linear + gelu (fp16 for logits stationary, bf16 for Qp natural),
        # then Qp natural via xbar on the Sync queue
        qs = slice(qg * 512, (qg + 1) * 512)
        for om in range(HC):
            plin = psum_l.tile([128, 512], F32, tag="pl", name=f"plin{qg}_{om}")
            for hc in range(HC):
                nc.tensor.matmul(
                    plin[:],
                    wt[:, om, hc, :],
                    qts[qg][:, :, hc, :],
                    start=(hc == 0),
                    stop=(hc == HC - 1),
                )
            nc.scalar.activation(qpt[:, om, qs], plin[:], AF.Gelu,
                                 bias=bias[:, om, :], scale=1.0)
            nc.scalar.activation(qpt_bf[:, om, qs], plin[:], AF.Gelu,
                                 bias=bias[:, om, :], scale=1.0)
        for om in range(HC):
            nc.sync.dma_start_transpose(
                qps[qg][:, :, om * 128:(om + 1) * 128], qpt_bf[:, om, qs])

    c_prep(0)
    qt_prep(0, nc.scalar)
    q_group(0)
    c_prep(1)

    # Attended lags logits+exp by LAG q-tiles so the PE never waits on the
    # ACT exp latency or the qp xbar-transpose chain.
    for dc in range(NDC):
        c_nat = c_nats[dc][0]
        ct = cts[dc]
        expt = exppool.tile([128, NQT, DC], BF16)
        pa = [psum_a.tile([128, H + 1], F32, tag=f"a{dm}", name=f"pa{dm}")
              for dm in range(NDM)]
        nxt = dc + 2
        tsteps = (15, 17) if dc == 0 else (8, 11)
        for step in range(NQT + LAG):
            if dc == 0 and step in (1, 5, 9):
                qt_prep(step // 4 + 1, nc.sync)
            if dc == 0 and step in (4, 8, 12):
                q_group(step // 4)
            if step == 2 and dc + 3 < NDC:
                c_nats[dc + 3] = c_load(dc + 3)
            if step == 6 and nxt < NDC and nxt not in cts:
                cts[nxt] = ct_alloc(nxt)
            if step in tsteps and nxt < NDC and nxt > 1:
                hc = 0 if step == tsteps[0] else 1
                pt = c_transpose(nxt, c_nats[nxt][1], hc)
                nc.vector.tensor_copy(cts[nxt][:, hc, :], pt[:])
            if step < NQT:
                qi = step
                if qi in (5, 13) and dc > 0:
                    pl = psum_t.tile([128, DC], F32, tag="pt", name=f"plx{dc}_{qi}")
                else:
                    pl = psum_l.tile([128, DC], F32, tag="pl")
                for hc in range(HC):
                    nc.tensor.matmul(
                        pl[:],
                        qpt[:, hc, qi * 128:(qi + 1) * 128],
                        ct[:, hc, :],
                        start=(hc == 0),
                        stop=(hc == HC - 1),
                    )
                nc.scalar.activation(expt[:, qi, :], pl[:], AF.Exp)
            if step >= LAG:
                qj = step - LAG
                for dm in range(NDM):
                    nc.tensor.matmul(
                        pa[dm][:],
                        expt[:, qj, dm * 128:(dm + 1) * 128],
                        qps[qj // 4][:, qj % 4, 0:H + 1],
                        start=(qj == 0),
                        stop=(qj == NQT - 1),
                    )

        o_sb = outpool.tile([128, NDM, H], F32)
        for dm in range(NDM):
            rec = small.tile([128, 1], F32)
            nc.vector.reciprocal(rec[:], pa[dm][:, H:H + 1])
            nc.vector.scalar_tensor_tensor(
                o_sb[:, dm, :], pa[dm][:, 0:H], rec[:], c_nat[:, dm, :],
                ALU.mult, ALU.add,
            )
        nc.sync.dma_start(
            out=Od[dc * DC:(dc + 1) * DC, :].rearrange("(a p) h -> p a h", p=128),
            in_=o_sb[:])
        del c_nats[dc], cts[dc]


def build_nc():
    nc = bacc.Bacc("TRN2", target_bir_lowering=False, debug=False,
                   num_devices=N_CORES)
    Qd = nc.dram_tensor("Q", [QL, H], F32, kind="ExternalInput")
    Cd = nc.dram_tensor("C", [D, H], F32, kind="ExternalInput")
    Wd = nc.dram_tensor("W", [H, H], F32, kind="ExternalInput")
    bd = nc.dram_tensor("b", [H], F32, kind="ExternalInput")
    Od = nc.dram_tensor("out", [D, H], F32, kind="ExternalOutput")
    with tile.TileContext(nc) as tc:
        with ExitStack() as ctx:
            build_body(ctx, tc, nc, Qd[:], Cd[:], Wd[:], bd[:], Od[:])
    nc.finalize()
    return nc


_NC = None


def get_nc():
    global _NC
    if _NC is None:
        _NC = build_nc()
    return _NC


def kernel(Q, C, W, b):
    assert Q.shape == (B, QL, H) and C.shape == (B, D, H)
    nc = get_nc()
    in_maps = [
        {
            "Q": np.ascontiguousarray(Q[i], dtype=np.float32),
            "C": np.ascontiguousarray(C[i], dtype=np.float32),
            "W": np.ascontiguousarray(W, dtype=np.float32),
            "b": np.ascontiguousarray(b, dtype=np.float32),
        }
        for i in range(N_CORES)
    ]
    res = run_bass_kernel_spmd(nc, in_maps, core_ids=list(range(N_CORES)))
    return np.stack([res.results[i]["out"] for i in range(N_CORES)], axis=0)


# revision 27
# speedup vs baseline: 1.1215x; 1.0271x over previous
"""CQAttention Trainium2 kernel.

Math (per batch b, H=256, q=2048, d=8192):
  Qp   = gelu(Q @ W.T + b)                       [q, H]
  S    = C @ Qp.T                                [d, q]
  P    = softmax(S, axis=q)
  out  = P @ Qp + C                              [d, H]

Sharding: data-parallel over batch, one batch per NeuronCore (8 cores).

Per-core pipeline (v6 = tuned baseline):
  - C^T via PE transposes (identity matmul) as in the baseline: the DMA
    xbar path for C^T loses ~30us to DMA-ring serialization + ACT FIFO
    head-of-line blocking (measured), PE transposes cost only ~7us.
  - W^T, Q^T, Qp-natural via DMA xbar transposes instead of the PE
    (setup-sized, off the critical path): frees ~7us of PE work and the
    whole setup_pt PSUM machinery.  HW-validated xbar out-AP patterns
    only: full-tile 3D/4D outs (mid dims fold into the logical row index,
    mid-major order) and 3D outs with 128-multiple strides/offsets.
  - Logits^T tiles [q=128, d=512] with fp16 operands (full PE rate,
    ~11-bit mantissa); exp on ACT straight from PSUM to bf16 (softmax
    without max-subtraction: |logits| < ~70 so fp32 exp is safe);
    attended accumulated over the 16 q-tiles into PSUM [d=128, 257]
    where column 256 is the row-sum (ones column on Qp); fused epilogue
    out = (attended * 1/rowsum) + C in one DVE op per tile.
  - Setup DMA order = critical-chain order (W, C0, Q0:4 first): the SP
    DMA ring serializes transfers at ~150-230 GB/s effective.
  - Chunk pipeline: C loads 3 chunks ahead, C transposes 2 chunks ahead,
    attended lags logits/exp by 3 steps; the Q-side setup is folded into
    chunk 0's step loop so the PE never idles at startup.
"""

from contextlib import ExitStack

import numpy as np

import concourse.mybir as mybir
import concourse.tile as tile
from concourse import bacc
from concourse.bass_utils import run_bass_kernel_spmd
from concourse.masks import make_identity

B, QL, D, H = 8, 2048, 8192, 256
N_CORES = 8
F32 = mybir.dt.float32
BF16 = mybir.dt.bfloat16
F16 = mybir.dt.float16

HC = H // 128      # feature chunks (2)
NQT = QL // 128    # q tiles (16)
NQG = NQT // 4     # q groups (4)
DC = 512           # d-chunk size
NDC = D // DC      # d chunks (16)
NDM = DC // 128    # d tiles per chunk (4)
QPW = 512          # qp row stride (xbar needs 128-multiple; col 256 = ones)
LAG = 3            # attended lag behind logits/exp

AF = mybir.ActivationFunctionType
ALU = mybir.AluOpType


def build_body(ctx: ExitStack, tc: tile.TileContext, nc, Qd, Cd, Wd, bd, Od):
    singles = ctx.enter_context(tc.tile_pool(name="singles", bufs=1))
    qstat = ctx.enter_context(tc.tile_pool(name="qstat", bufs=1))
    cpool = ctx.enter_context(tc.tile_pool(name="cpool", bufs=5))
    ctpool = ctx.enter_context(tc.tile_pool(name="ctp", bufs=3))
    exppool = ctx.enter_context(tc.tile_pool(name="expp", bufs=2))
    outpool = ctx.enter_context(tc.tile_pool(name="outp", bufs=3))
    small = ctx.enter_context(tc.tile_pool(name="small", bufs=4))
    psum_l = ctx.enter_context(tc.tile_pool(name="psl", bufs=2, space="PSUM"))
    psum_t = ctx.enter_context(tc.tile_pool(name="pst", bufs=2, space="PSUM"))
    psum_a = ctx.enter_context(tc.tile_pool(name="psa", bufs=1, space="PSUM"))

    ident = singles.tile([128, 128], F16)
    make_identity(nc, ident)

    # --- C path ---
    c_nats = {}
    cts = {}

    def c_load(dc, cast=True):
        c_nat = cpool.tile([128, NDM, H], F32, tag="cnat", name=f"cnat{dc}")
        nc.sync.dma_start(
            out=c_nat[:],
            in_=Cd[dc * DC:(dc + 1) * DC, :].rearrange("(a p) h -> p a h", p=128))
        if not cast:
            return c_nat
        c_bf = cpool.tile([128, NDM, H], F16, tag="cbf", name=f"cbf{dc}")
        nc.vector.tensor_copy(c_bf[:], c_nat[:])
        return c_nat, c_bf

    def c_cast(dc, c_nat):
        c_bf = cpool.tile([128, NDM, H], F16, tag="cbf", name=f"cbf{dc}")
        nc.vector.tensor_copy(c_bf[:], c_nat[:])
        return c_nat, c_bf

    def c_transpose(dc, c_bf, hc):
        pt = psum_t.tile([128, 512], F16, tag="pt", name=f"ptc{dc}_{hc}")
        for dm in range(NDM):
            nc.tensor.transpose(
                pt[:, dm * 128:(dm + 1) * 128],
                c_bf[:, dm, hc * 128:(hc + 1) * 128], ident[:])
        return pt

    def ct_alloc(dc):
        return ctpool.tile([128, HC, DC], F16, tag="ct", name=f"ct{dc}")

    def c_prep(dc):
        cts[dc] = ct_alloc(dc)
        for hc in range(HC):
            pt = c_transpose(dc, c_nats[dc][1], hc)
            nc.vector.tensor_copy(cts[dc][:, hc, :], pt[:])

    # --- static tiles ---
    w_nat = singles.tile([128, HC, H], F32)  # [o in-chunk, om, h]
    w_f16 = singles.tile([128, HC, H], F16, name="w_f16")
    # wt[h', om, hc, o'] = W[om*128+o', hc*128+h']
    wt = qstat.tile([128, HC, HC, 128], F16)
    bias = singles.tile([128, HC, 1], F32)
    q_nat = cpool.tile([128, NQT, H], F32, tag="qnat", bufs=1)
    q_f16 = cpool.tile([128, NQT, H], F16, tag="qf16", bufs=1)
    # qt[g][h', k, hc, q'] = Q[(4g+k)*128+q', hc*128+h']
    qts = [qstat.tile([128, 4, HC, 128], F16, name=f"qt{g}") for g in range(NQG)]
    # qp[g][q', k, col] = Qp[(4g+k)*128+q', col], col 256 = 1.0
    qps = [qstat.tile([128, 4, QPW], BF16, name=f"qp{g}") for g in range(NQG)]
    qpt = qstat.tile([128, HC, QL], F16)
    qpt_bf = qstat.tile([128, HC, QL], BF16)
    for g in range(NQG):
        nc.vector.memset(qps[g][:, :, H:H + 1], 1.0)

    # --- setup loads: critical-chain order on the serialized SP ring.
    # Q comes in 4 group-aligned pieces so qt transposes start per-group. ---
    q_view = Qd.rearrange("(a p) h -> p a h", p=128)

    def load_q(g):
        nc.sync.dma_start(out=q_nat[:, g * 4:(g + 1) * 4, :],
                          in_=q_view[:, g * 4:(g + 1) * 4, :])

    def cast_q(g):
        nc.vector.tensor_copy(q_f16[:, g * 4:(g + 1) * 4, :],
                              q_nat[:, g * 4:(g + 1) * 4, :])

    nc.sync.dma_start(out=w_nat[:],
                      in_=Wd.rearrange("(a p) h -> p a h", p=128))
    nc.sync.dma_start(out=bias[:, :, 0], in_=bd.rearrange("(c p) -> p c", p=128))
    load_q(0)
    cn0 = c_load(0, cast=False)
    load_q(1)
    load_q(2)
    load_q(3)
    cn1 = c_load(1, cast=False)
    cn2 = c_load(2, cast=False)
    cn3 = c_load(3, cast=False)

    nc.vector.tensor_copy(w_f16[:], w_nat[:])
    cast_q(0)
    c_nats[0] = c_cast(0, cn0)
    cast_q(1)
    cast_q(2)
    cast_q(3)
    c_nats[1] = c_cast(1, cn1)
    c_nats[2] = c_cast(2, cn2)
    c_nats[3] = c_cast(3, cn3)

    # setup transposes on ACT's hwdge queue (idle pre-loop)
    nc.scalar.dma_start_transpose(wt[:], w_f16[:])

    def qt_prep(qg, eng):
        eng.dma_start_transpose(qts[qg][:], q_f16[:, qg * 4:(qg + 1) * 4, :])

    def q_group(qg):
        # linear + gelu (fp16 for logits stationary, bf16 for Qp natural),
        # then Qp natural via xbar on the Sync queue
        qs = slice(qg * 512, (qg + 1) * 512)
        for om in range(HC):
            plin = psum_l.tile([128, 512], F32, tag="pl", name=f"plin{qg}_{om}")
            for hc in range(HC):
                nc.tensor.matmul(
                    plin[:],
                    wt[:, om, hc, :],
                    qts[qg][:, :, hc, :],
                    start=(hc == 0),
                    stop=(hc == HC - 1),
                )
            nc.scalar.activation(qpt[:, om, qs], plin[:], AF.Gelu,
                                 bias=bias[:, om, :], scale=1.0)
            nc.scalar.activation(qpt_bf[:, om, qs], plin[:], AF.Gelu,
                                 bias=bias[:, om, :], scale=1.0)
        for om in range(HC):
            nc.sync.dma_start_transpose(
                qps[qg][:, :, om * 128:(om + 1) * 128], qpt_bf[:, om, qs])

    qt_prep(0, nc.scalar)
    q_group(0)
    c_prep(0)

    # Attended lags logits+exp by LAG q-tiles so the PE never waits on the
    # ACT exp latency or the qp xbar-transpose chain.
    for dc in range(NDC):
        c_nat = c_nats[dc][0]
        ct = cts[dc]
        expt = exppool.tile([128, NQT, DC], BF16)
        pa = [psum_a.tile([128, H + 1], F32, tag=f"a{dm}", name=f"pa{dm}")
              for dm in range(NDM)]
        nxt = dc + 2
        tsteps = (15, 17) if dc == 0 else (8, 11)
        for step in range(NQT + LAG):
            if dc == 0 and step in (1, 5, 9):
                qt_prep(step // 4 + 1, nc.sync)
            if dc == 0 and step in (4, 8, 12):
                q_group(step // 4)
            if dc == 0 and step in (6, 8):
                hc = 0 if step == 6 else 1
                if step == 6:
                    cts[1] = ct_alloc(1)
                pt = c_transpose(1, c_nats[1][1], hc)
                nc.vector.tensor_copy(cts[1][:, hc, :], pt[:])
            if step == 2 and 4 <= dc + 3 < NDC:
                c_nats[dc + 3] = c_load(dc + 3)
            if step == 6 and nxt < NDC and nxt not in cts:
                cts[nxt] = ct_alloc(nxt)
            if step in tsteps and nxt < NDC and nxt > 1:
                hc = 0 if step == tsteps[0] else 1
                pt = c_transpose(nxt, c_nats[nxt][1], hc)
                nc.vector.tensor_copy(cts[nxt][:, hc, :], pt[:])
            if step < NQT:
                qi = step
                if qi in (5, 13) and dc > 0:
                    pl = psum_t.tile([128, DC], F32, tag="pt", name=f"plx{dc}_{qi}")
                else:
                    pl = psum_l.tile([128, DC], F32, tag="pl")
                for hc in range(HC):
                    nc.tensor.matmul(
                        pl[:],
                        qpt[:, hc, qi * 128:(qi + 1) * 128],
                        ct[:, hc, :],
                        start=(hc == 0),
                        stop=(hc == HC - 1),
                    )
                nc.scalar.activation(expt[:, qi, :], pl[:], AF.Exp)
            if step >= LAG:
                qj = step - LAG
                for dm in range(NDM):
                    nc.tensor.matmul(
                        pa[dm][:],
                        expt[:, qj, dm * 128:(dm + 1) * 128],
                        qps[qj // 4][:, qj % 4, 0:H + 1],
                        start=(qj == 0),
                        stop=(qj == NQT - 1),
                    )

        o_sb = outpool.tile([128, NDM, H], F32)
        for dm in range(NDM):
            rec = small.tile([128, 1], F32)
            nc.vector.reciprocal(rec[:], pa[dm][:, H:H + 1])
            nc.vector.scalar_tensor_tensor(
                o_sb[:, dm, :], pa[dm][:, 0:H], rec[:], c_nat[:, dm, :],
                ALU.mult, ALU.add,
            )
        nc.sync.dma_start(
            out=Od[dc * DC:(dc + 1) * DC, :].rearrange("(a p) h -> p a h", p=128),
            in_=o_sb[:])
        del c_nats[dc], cts[dc]


def build_nc():
    nc = bacc.Bacc("TRN2", target_bir_lowering=False, debug=False,
                   num_devices=N_CORES)
    Qd = nc.dram_tensor("Q", [QL, H], F32, kind="ExternalInput")
    Cd = nc.dram_tensor("C", [D, H], F32, kind="ExternalInput")
    Wd = nc.dram_tensor("W", [H, H], F32, kind="ExternalInput")
    bd = nc.dram_tensor("b", [H], F32, kind="ExternalInput")
    Od = nc.dram_tensor("out", [D, H], F32, kind="ExternalOutput")
    with tile.TileContext(nc) as tc:
        with ExitStack() as ctx:
            build_body(ctx, tc, nc, Qd[:], Cd[:], Wd[:], bd[:], Od[:])
    nc.finalize()
    return nc


_NC = None


def get_nc():
    global _NC
    if _NC is None:
        _NC = build_nc()
    return _NC


def kernel(Q, C, W, b):
    assert Q.shape == (B, QL, H) and C.shape == (B, D, H)
    nc = get_nc()
    in_maps = [
        {
            "Q": np.ascontiguousarray(Q[i], dtype=np.float32),
            "C": np.ascontiguousarray(C[i], dtype=np.float32),
            "W": np.ascontiguousarray(W, dtype=np.float32),
            "b": np.ascontiguousarray(b, dtype=np.float32),
        }
        for i in range(N_CORES)
    ]
    res = run_bass_kernel_spmd(nc, in_maps, core_ids=list(range(N_CORES)))
    return np.stack([res.results[i]["out"] for i in range(N_CORES)], axis=0)
